# revision 46
# baseline (speedup 1.0000x reference)
"""Trainium2 Bass kernel for HarmonicDDSPEngine.

Strategy v3 (pure batch sharding, zero cross-core communication):
  - Each core owns 2 batches x full T. The sin table is compressed via the
    angle-addition identity: sin(c_k*(j*L + n)) = sin(theta_jk)*cos(phi_kn)
    + cos(theta_jk)*sin(phi_kn), so the on-device table is a core-INDEPENDENT
    (128, L) fp16 [cos;sin] stack (706 KB) and all per-core variation moves
    into a (128,128) fp16 weight matrix W computed on host:
        W[k, p=(b,j)]    = A[b,k]*sin(theta_jk)
        W[64+k, p=(b,j)] = A[b,k]*cos(theta_jk)
    harmonics = W^T @ [cos;sin] in one PSUM accumulation step per tile.
  - The noise add rides the PSUM accumulation: noise is shipped as
    (noise - 0.5) fp16 and multiplied by a per-batch diag(2*lev) "identity"
    in a second accumulation matmul, so no separate nsig pass is needed.
  - Per-batch abs-max is fully local (2 batches per core): per-tile reduce ->
    (128,1) fold -> PE transpose (fp16) -> (1,128) -> j-fold -> (1,2) -> +eps
    -> recip -> broadcast-copy -> PE matmul x ones -> (128,1) -> normalize.
  - No collectives, no remote DMA: every core's execution is independent, so
    launch skew between cores cannot inflate any core's execution window.
  - Envelope * gain lattice (exact at integer sample points), fp16, in two
    halves with per-half affine biases off a half-width fp16 iota:
      att  = sc_att*i + bi_att           (DVE dual-op; its relu is a no-op)
      z    = relu(sc_z*i + bi_z)         (ACT)
      decs = sc_d2*z + bi_d2             (ACT h0 / DVE h1)
      wu   = relu(sc_w*i + bi_w)         (ACT)
      envg = relu(min(att, decs) - wu)   (DVE min/sub + ACT h0 / DVE h1 relu)

Accuracy note: the reference quantizes sin arguments to fp32 (args up to
~7e5 rad), which a rank-1 angle split cannot reproduce pointwise; measured
end-to-end rel_l2 vs the reference is ~8e-3 (gate: 2e-2).
"""

import os
import numpy as np

import concourse.bacc as bacc
import concourse.mybir as mybir
import concourse.tile as tile
from concourse.bass_utils import run_bass_kernel_spmd

F32 = mybir.dt.float32
F16 = mybir.dt.float16
f32 = np.float32
f16 = np.float16

B, T, NH = 16, 176400, 64
SR = 44100
NCORES = 8
BL = 2            # batches per core
J = 64            # t-subblocks per batch
L = 2760          # samples per subblock
H2 = L // 2       # envelope half width
TPAD = J * L      # 176640
NT = 6            # PSUM tiles per core
N = L // NT       # 460, fits one PSUM bank
NS2 = 3           # table DMA chunks

_cache = {}


def _build_nc():
    nc = bacc.Bacc(None, num_devices=NCORES)

    tab_d = nc.dram_tensor("tab", [128, L], F16, kind="ExternalInput")
    w_d = nc.dram_tensor("wmat", [128, 128], F16, kind="ExternalInput")
    ident_d = nc.dram_tensor("ident", [128, 128], F16, kind="ExternalInput")
    identp_d = nc.dram_tensor("identp", [128, 128], F16, kind="ExternalInput")
    noise_d = nc.dram_tensor("noise_p", [128, L], F16, kind="ExternalInput")
    consts_d = nc.dram_tensor("consts", [128, 16], F32, kind="ExternalInput")
    out_d = nc.dram_tensor("out_sig", [128, L], F16, kind="ExternalOutput")

    AF = mybir.ActivationFunctionType
    OP = mybir.AluOpType

    with tile.TileContext(nc) as tc:
        with (
            tc.tile_pool(name="const", bufs=1) as cpool,
            tc.tile_pool(name="env", bufs=12) as epool,
            tc.tile_pool(name="sig", bufs=1) as spool,
            tc.tile_pool(name="small", bufs=8) as smpool,
            tc.tile_pool(name="psum", bufs=NT, space="PSUM") as ppool,
            tc.tile_pool(name="psb", bufs=2, space="PSUM") as pbpool,
        ):
            # tiny dummy ACT so the auto-inserted ACT table load runs during
            # the DMA window instead of gating the first envelope pass
            tiny = smpool.tile([128, 1], F32, tag="tiny")
            nc.vector.memset(tiny[:], 0.0)
            nc.scalar.activation(tiny[:], tiny[:], mybir.ActivationFunctionType.Relu)

            # half-width fp16 iota (0..1379 exact; per-half affine biases)
            iot = cpool.tile([128, H2], F16, tag="iot")
            nc.gpsimd.iota(iot[:], [[1, H2]], base=0, channel_multiplier=0,
                           allow_small_or_imprecise_dtypes=True)

            # ---- input DMAs (HWDGE) ----
            consts = cpool.tile([128, 16], F32, tag="consts")
            nc.sync.dma_start(consts[:], consts_d[:])
            tab = cpool.tile([128, L], F16, tag="tab")
            NC2 = L // NS2
            nc.sync.dma_start(tab[:, 0:NC2], tab_d[:, 0:NC2])
            noise_t = cpool.tile([128, L], F16, tag="noise_t")
            nc.sync.dma_start(noise_t[:], noise_d[:])
            wmat = cpool.tile([128, 128], F16, tag="wmat")
            nc.sync.dma_start(wmat[:], w_d[:])
            ident = cpool.tile([128, 128], F16, tag="ident")
            nc.sync.dma_start(ident[:], ident_d[:])
            for s2 in range(1, NS2):
                sl = slice(s2 * NC2, (s2 + 1) * NC2)
                nc.sync.dma_start(tab[:, sl], tab_d[:, sl])
            identp = cpool.tile([128, 128], F16, tag="identp")
            nc.sync.dma_start(identp[:], identp_d[:])

            def cst(i):
                return consts[:, i:i + 1]

            # ---- absorber matmuls: pull DMA waits onto PE early ----
            scr = pbpool.tile([128, 1], F32, tag="ps2", name="scr")
            nc.tensor.matmul(scr[:], wmat[:], wmat[:, 0:1],
                             start=True, stop=True)
            nc.tensor.matmul(scr[:], ident[:], ident[:, 0:1],
                             start=True, stop=True)

            # ---- envelope * gain (exact lattice), fp16, two halves ----
            # att has no active relu (its affine is >= 0 for i >= 0), so it
            # runs on DVE as a dual-op tensor_scalar; z/decs/wu stay on ACT.
            envgs = []
            for h in range(2):
                att = epool.tile([128, H2], F16, tag="env", name=f"att{h}")
                nc.vector.tensor_scalar(att[:], iot[:], cst(0), cst(1 + h),
                                        OP.mult, OP.add)
                z = epool.tile([128, H2], F16, tag="env", name=f"z{h}")
                nc.scalar.activation(z[:], iot[:], AF.Relu,
                                     bias=cst(4 + h), scale=cst(3))
                decs = epool.tile([128, H2], F16, tag="env", name=f"decs{h}")
                if h == 0:
                    nc.scalar.activation(decs[:], z[:], AF.Identity,
                                         bias=cst(7), scale=cst(6))
                else:
                    # unload ACT's critical tail: decs1 on DVE (fp16 2x rate)
                    nc.vector.tensor_scalar(decs[:], z[:], cst(6), cst(7),
                                            OP.mult, OP.add)
                wu = epool.tile([128, H2], F16, tag="env", name=f"wu{h}")
                nc.scalar.activation(wu[:], iot[:], AF.Relu,
                                     bias=cst(9 + h), scale=cst(8))
                mm = epool.tile([128, H2], F16, tag="env", name=f"mm{h}")
                nc.vector.tensor_tensor(mm[:], att[:], decs[:], OP.min)
                env0 = epool.tile([128, H2], F16, tag="env", name=f"env0{h}")
                nc.vector.tensor_tensor(env0[:], mm[:], wu[:], OP.subtract)
                envg = cpool.tile([128, H2], F16, tag=f"envg{h}",
                                  name=f"envg{h}")
                if h == 0:
                    nc.scalar.activation(envg[:], env0[:], AF.Relu)
                else:
                    nc.vector.tensor_scalar(envg[:], env0[:], 0.0, None,
                                            OP.max)
                envgs.append(envg)

            # ---- harmonics matmuls (+ fused noise add) + signal chain ----
            # ident carries diag(2*lev_b) so the noise term (noise-0.5)*2lev
            # is accumulated by PE directly.
            sig = spool.tile([128, L], F16, tag="sig")
            outn = spool.tile([128, L], F16, tag="outn")
            mxcols = smpool.tile([128, NT], F16, tag="mxc")
            psums = [ppool.tile([128, N], F32, tag="ps", name=f"ps{i}")
                     for i in range(NT)]
            for s in range(NT):
                ps = psums[s]
                sl = slice(s * N, (s + 1) * N)
                nc.tensor.matmul(ps[:], wmat[:], tab[:, sl],
                                 start=True, stop=False)
                nc.tensor.matmul(ps[:], ident[:], noise_t[:, sl],
                                 start=False, stop=True)
                eh = envgs[s // 3]
                el = slice((s % 3) * N, (s % 3 + 1) * N)
                nc.vector.tensor_tensor(sig[:, sl], ps[:], eh[:, el], OP.mult)
                nc.vector.tensor_reduce(mxcols[:, s:s + 1], sig[:, sl],
                                        axis=mybir.AxisListType.X, op=OP.max,
                                        apply_absolute_value=True)

            # ---- per-batch max: fold -> transpose -> j-fold -> bcast ----
            mx = smpool.tile([128, 1], F16, tag="mx")
            nc.vector.tensor_reduce(mx[:], mxcols[:], axis=mybir.AxisListType.X,
                                    op=OP.max)
            mxT = pbpool.tile([1, 128], F16, tag="ps2", name="mxT")
            nc.tensor.transpose(mxT[:], mx[:], identp[:])
            row2 = smpool.tile([1, 2], F32, tag="row2")
            nc.vector.tensor_reduce(row2[:],
                                    mxT[:].rearrange("o (b j) -> o b j", j=J),
                                    axis=mybir.AxisListType.X, op=OP.max)
            nc.vector.tensor_scalar(row2[:], row2[:], 1e-5, None, OP.add)
            inv2 = smpool.tile([1, 2], F32, tag="inv2")
            nc.vector.reciprocal(inv2[:], row2[:])
            invrow = smpool.tile([1, 128], F32, tag="invrow")
            nc.vector.tensor_copy(
                invrow[:].rearrange("o (b j) -> o b j", j=J),
                inv2[:].rearrange("o (b u) -> o b u", u=1).broadcast_to(
                    [1, BL, J]))
            invp = pbpool.tile([128, 1], F32, tag="ps2", name="invp")
            nc.tensor.matmul(invp[:], invrow[:], consts[0:1, 13:14],
                             start=True, stop=True)
            inv = smpool.tile([128, 1], F32, tag="inv")
            nc.vector.tensor_copy(inv[:], invp[:])

            # ---- normalize + store (DVE half || ACT half, 2 DMAs) ----
            nc.vector.tensor_scalar(outn[:, 0:H2], sig[:, 0:H2], inv[:],
                                    None, OP.mult)
            nc.sync.dma_start(out_d[:, 0:H2], outn[:, 0:H2])
            nc.vector.tensor_scalar(outn[:, H2:L], sig[:, H2:L], inv[:],
                                    None, OP.mult)
            nc.sync.dma_start(out_d[:, H2:L], outn[:, H2:L])

    nc.finalize()
    return nc


def _host_prep(harmonic_dist, noise_bands, adsr, gain, noise):
    """Weights/consts in f64 (cast f32/f16 at the end); the angle split is
    sin(theta_jk + phi_kn) with both angles exact in f64."""
    step64 = np.float64(f32(np.float64(T / SR) / (T - 1)))
    k = np.arange(1, NH + 1, dtype=f32)
    ck64 = (f32(2.0 * np.pi * 440.0) * k).astype(np.float64)
    n = np.arange(L, dtype=np.float64)
    jj = np.arange(J, dtype=np.float64)
    phi = ck64[:, None] * (step64 * n[None, :])          # (64, L)
    theta = ck64[:, None] * (step64 * (jj[None, :] * L))  # (64, J)
    tab = np.concatenate([np.cos(phi), np.sin(phi)], axis=0).astype(f16)
    sinth, costh = np.sin(theta), np.cos(theta)          # (64, J)

    A = np.ascontiguousarray(harmonic_dist, dtype=f32).astype(np.float64)
    identp = np.eye(128, dtype=f16)

    # noise shipped as (noise - 0.5) fp16; the 2*lev scale rides in the
    # per-batch scaled identity so no on-device nsig pass is needed
    npad = np.zeros((B, TPAD), f16)
    npad[:, :T] = (noise.astype(f32) - f32(0.5)).astype(f16)

    # ADSR int constants, replicating reference rounding exactly
    att_in, dec_in, sus, rel_in = (adsr[:, 0].astype(f32), adsr[:, 1].astype(f32),
                                   adsr[:, 2].astype(f32), adsr[:, 3].astype(f32))
    a = np.floor((att_in * f32(0.5)) * f32(SR)).astype(np.int64) + 1
    d = np.floor((dec_in * f32(0.5)) * f32(SR)).astype(np.int64) + 1
    r = np.floor((rel_in * f32(0.5)) * f32(SR)).astype(np.int64) + 1
    total = a + d + r
    scale = (f32(T) / total.astype(f32)).astype(f32)
    resc = total > T
    a = np.where(resc, np.floor(a.astype(f32) * scale).astype(np.int64), a)
    d = np.where(resc, np.floor(d.astype(f32) * scale).astype(np.int64), d)
    r = np.where(resc, np.floor(r.astype(f32) * scale).astype(np.int64), r)
    s = np.maximum(T - (a + d + r), 0)

    g64 = gain.astype(np.float64)[:, 0]
    sus64 = sus.astype(np.float64)
    m_a = np.maximum(a - 1, 1).astype(np.float64)
    m_d = np.maximum(d - 1, 1).astype(np.float64)
    m_r = np.maximum(r - 1, 1).astype(np.float64)
    A2 = (a + d + s).astype(np.float64)
    lev64 = (np.mean(noise_bands.astype(f32), axis=1, dtype=f32)
             * f32(0.1)).astype(np.float64)

    in_maps = []
    for c in range(NCORES):
        noise_c = np.ascontiguousarray(
            npad[2 * c:2 * c + 2].reshape(128, L))

        sident = np.zeros((128, 128), np.float64)
        for p in range(128):
            sident[p, p] = 2.0 * lev64[2 * c + p // J]

        wmat = np.zeros((128, 128), np.float64)
        for bl in range(BL):
            b = 2 * c + bl
            # W[k, p] = A[b,k]*sin(theta[k,j]); W[64+k, p] = A[b,k]*cos(..)
            wmat[:NH, bl * J:(bl + 1) * J] = A[b][:, None] * sinth
            wmat[NH:, bl * J:(bl + 1) * J] = A[b][:, None] * costh

        consts = np.zeros((128, 16), np.float64)
        consts[:, 13] = 1.0
        for bl in range(BL):
            b = 2 * c + bl
            for j in range(J):
                p = bl * J + j
                base = j * L
                sc_att = g64[b] / m_a[b]
                consts[p, 0] = sc_att
                consts[p, 1] = f32(sc_att) * np.float64(base)
                consts[p, 2] = f32(sc_att) * np.float64(base + H2)
                sc_z = -1.0 / m_d[b]
                consts[p, 3] = sc_z
                consts[p, 4] = 1.0 - (base - a[b]) / m_d[b]
                consts[p, 5] = 1.0 - (base + H2 - a[b]) / m_d[b]
                consts[p, 6] = (1.0 - sus64[b]) * g64[b]
                consts[p, 7] = sus64[b] * g64[b]
                sc_w = sus64[b] * g64[b] / m_r[b]
                consts[p, 8] = sc_w
                consts[p, 9] = -f32(sc_w) * (A2[b] - np.float64(base))
                consts[p, 10] = -f32(sc_w) * (A2[b] - np.float64(base + H2))
                consts[p, 11] = 2.0 * lev64[b]
                consts[p, 12] = -lev64[b]
        in_maps.append({
            "tab": tab,
            "wmat": wmat.astype(f16),
            "ident": sident.astype(f16),
            "identp": identp,
            "noise_p": noise_c,
            "consts": consts.astype(f32),
        })
    return in_maps


LAST_RESULTS = None


def kernel(base_audio, harmonic_dist, noise_bands, adsr, gain, noise):
    global LAST_RESULTS
    if "nc" not in _cache:
        _cache["nc"] = _build_nc()
    nc = _cache["nc"]

    in_maps = _host_prep(
        np.asarray(harmonic_dist), np.asarray(noise_bands),
        np.asarray(adsr), np.asarray(gain), np.asarray(noise))

    trace = bool(os.environ.get("KERNEL_TRACE"))
    res = run_bass_kernel_spmd(nc, in_maps, list(range(NCORES)), trace=trace)
    LAST_RESULTS = res

    out = np.empty((B, TPAD), f32)
    for c in range(NCORES):
        out[2 * c:2 * c + 2] = (res.results[c]["out_sig"]
                                .astype(f32).reshape(BL, TPAD))
    return np.ascontiguousarray(out[:, :T])


# revision 47
# speedup vs baseline: 1.0494x; 1.0494x over previous
"""Trainium2 Bass kernel for HarmonicDDSPEngine.

Strategy v3 (pure batch sharding, zero cross-core communication):
  - Each core owns 2 batches x full T. The sin table is compressed via the
    angle-addition identity: sin(c_k*(j*L + n)) = sin(theta_jk)*cos(phi_kn)
    + cos(theta_jk)*sin(phi_kn), so the on-device table is a core-INDEPENDENT
    (128, L) fp16 [cos;sin] stack (706 KB) and all per-core variation moves
    into a (128,128) fp16 weight matrix W computed on host:
        W[k, p=(b,j)]    = A[b,k]*sin(theta_jk)
        W[64+k, p=(b,j)] = A[b,k]*cos(theta_jk)
    harmonics = W^T @ [cos;sin] in one PSUM accumulation step per tile.
  - The noise add rides the PSUM accumulation: noise is shipped as
    (noise - 0.5) fp16 and multiplied by a per-batch diag(2*lev) "identity"
    in a second accumulation matmul, so no separate nsig pass is needed.
  - Per-batch abs-max is fully local (2 batches per core): per-tile reduce ->
    (128,1) fold -> PE transpose (fp16) -> (1,128) -> j-fold -> (1,2) -> +eps
    -> recip -> broadcast-copy -> PE matmul x ones -> (128,1) -> normalize.
  - No collectives, no remote DMA: every core's execution is independent, so
    launch skew between cores cannot inflate any core's execution window.
  - Envelope * gain lattice (exact at integer sample points), fp16, in two
    halves with per-half affine biases off a half-width fp16 iota:
      att  = sc_att*i + bi_att           (DVE dual-op; its relu is a no-op)
      z    = relu(sc_z*i + bi_z)         (ACT)
      decs = sc_d2*z + bi_d2             (ACT h0 / DVE h1)
      wu   = relu(sc_w*i + bi_w)         (ACT)
      envg = relu(min(att, decs) - wu)   (DVE min/sub + ACT h0 / DVE h1 relu)

Accuracy note: the reference quantizes sin arguments to fp32 (args up to
~7e5 rad), which a rank-1 angle split cannot reproduce pointwise; measured
end-to-end rel_l2 vs the reference is ~8e-3 (gate: 2e-2).
"""

import os
import numpy as np

import concourse.bacc as bacc
import concourse.mybir as mybir
import concourse.tile as tile
from concourse.bass_utils import run_bass_kernel_spmd

F32 = mybir.dt.float32
F16 = mybir.dt.float16
f32 = np.float32
f16 = np.float16

B, T, NH = 16, 176400, 64
SR = 44100
NCORES = 8
BL = 2            # batches per core
J = 64            # t-subblocks per batch
L = 2760          # samples per subblock
H2 = L // 2       # envelope half width
TPAD = J * L      # 176640
NT = 6            # PSUM tiles per core
N = L // NT       # 460, fits one PSUM bank
NS2 = 3           # table DMA chunks

_cache = {}


def _build_nc():
    nc = bacc.Bacc(None, num_devices=NCORES)

    tab_d = nc.dram_tensor("tab", [128, L], F16, kind="ExternalInput")
    w_d = nc.dram_tensor("wmat", [128, 128], F16, kind="ExternalInput")
    ident_d = nc.dram_tensor("ident", [128, 128], F16, kind="ExternalInput")
    identp_d = nc.dram_tensor("identp", [128, 128], F16, kind="ExternalInput")
    noise_d = nc.dram_tensor("noise_p", [128, L], F16, kind="ExternalInput")
    consts_d = nc.dram_tensor("consts", [128, 16], F32, kind="ExternalInput")
    out_d = nc.dram_tensor("out_sig", [128, L], F16, kind="ExternalOutput")

    AF = mybir.ActivationFunctionType
    OP = mybir.AluOpType

    with tile.TileContext(nc) as tc:
        with (
            tc.tile_pool(name="const", bufs=1) as cpool,
            tc.tile_pool(name="env", bufs=12) as epool,
            tc.tile_pool(name="sig", bufs=1) as spool,
            tc.tile_pool(name="small", bufs=8) as smpool,
            tc.tile_pool(name="psum", bufs=NT, space="PSUM") as ppool,
            tc.tile_pool(name="psb", bufs=2, space="PSUM") as pbpool,
        ):
            # tiny dummy ACT so the auto-inserted ACT table load runs during
            # the DMA window instead of gating the first envelope pass
            tiny = smpool.tile([128, 1], F32, tag="tiny")
            nc.vector.memset(tiny[:], 0.0)
            nc.scalar.activation(tiny[:], tiny[:], mybir.ActivationFunctionType.Relu)

            # half-width fp16 iota (0..1379 exact; per-half affine biases)
            iot = cpool.tile([128, H2], F16, tag="iot")
            nc.gpsimd.iota(iot[:], [[1, H2]], base=0, channel_multiplier=0,
                           allow_small_or_imprecise_dtypes=True)

            # ---- input DMAs (HWDGE) ----
            consts = cpool.tile([128, 16], F32, tag="consts")
            nc.sync.dma_start(consts[:], consts_d[:])
            tab = cpool.tile([128, L], F16, tag="tab")
            NC2 = L // NS2
            nc.sync.dma_start(tab[:, 0:NC2], tab_d[:, 0:NC2])
            noise_t = cpool.tile([128, L], F16, tag="noise_t")
            nc.sync.dma_start(noise_t[:], noise_d[:])
            wmat = cpool.tile([128, 128], F16, tag="wmat")
            nc.sync.dma_start(wmat[:], w_d[:])
            ident = cpool.tile([128, 128], F16, tag="ident")
            nc.sync.dma_start(ident[:], ident_d[:])
            for s2 in range(1, NS2):
                sl = slice(s2 * NC2, (s2 + 1) * NC2)
                nc.sync.dma_start(tab[:, sl], tab_d[:, sl])
            identp = cpool.tile([128, 128], F16, tag="identp")
            nc.sync.dma_start(identp[:], identp_d[:])

            def cst(i):
                return consts[:, i:i + 1]

            # ---- absorber matmuls: pull DMA waits onto PE early ----
            scr = pbpool.tile([128, 1], F32, tag="ps2", name="scr")
            nc.tensor.matmul(scr[:], wmat[:], wmat[:, 0:1],
                             start=True, stop=True)
            nc.tensor.matmul(scr[:], ident[:], ident[:, 0:1],
                             start=True, stop=True)

            # ---- envelope * gain (exact lattice), fp16, two halves ----
            # att has no active relu (its affine is >= 0 for i >= 0), so it
            # runs on DVE as a dual-op tensor_scalar; z/decs/wu stay on ACT.
            envgs = []
            for h in range(2):
                att = epool.tile([128, H2], F16, tag="env", name=f"att{h}")
                nc.vector.tensor_scalar(att[:], iot[:], cst(0), cst(1 + h),
                                        OP.mult, OP.add)
                z = epool.tile([128, H2], F16, tag="env", name=f"z{h}")
                nc.scalar.activation(z[:], iot[:], AF.Relu,
                                     bias=cst(4 + h), scale=cst(3))
                decs = epool.tile([128, H2], F16, tag="env", name=f"decs{h}")
                # decs on DVE (fp16 dual-op): fills DVE's early idle window
                # and shortens ACT's serial chain that gates the h1 lattice
                nc.vector.tensor_scalar(decs[:], z[:], cst(6), cst(7),
                                        OP.mult, OP.add)
                wu = epool.tile([128, H2], F16, tag="env", name=f"wu{h}")
                nc.scalar.activation(wu[:], iot[:], AF.Relu,
                                     bias=cst(9 + h), scale=cst(8))
                mm = epool.tile([128, H2], F16, tag="env", name=f"mm{h}")
                nc.vector.tensor_tensor(mm[:], att[:], decs[:], OP.min)
                env0 = epool.tile([128, H2], F16, tag="env", name=f"env0{h}")
                nc.vector.tensor_tensor(env0[:], mm[:], wu[:], OP.subtract)
                envg = cpool.tile([128, H2], F16, tag=f"envg{h}",
                                  name=f"envg{h}")
                if h == 0:
                    nc.scalar.activation(envg[:], env0[:], AF.Relu)
                else:
                    nc.vector.tensor_scalar(envg[:], env0[:], 0.0, None,
                                            OP.max)
                envgs.append(envg)

            # ---- harmonics matmuls (+ fused noise add) + signal chain ----
            # ident carries diag(2*lev_b) so the noise term (noise-0.5)*2lev
            # is accumulated by PE directly.
            sig = spool.tile([128, L], F16, tag="sig")
            outn = spool.tile([128, L], F16, tag="outn")
            mxcols = smpool.tile([128, NT], F16, tag="mxc")
            psums = [ppool.tile([128, N], F32, tag="ps", name=f"ps{i}")
                     for i in range(NT)]
            for s in range(NT):
                ps = psums[s]
                sl = slice(s * N, (s + 1) * N)
                nc.tensor.matmul(ps[:], wmat[:], tab[:, sl],
                                 start=True, stop=False)
                nc.tensor.matmul(ps[:], ident[:], noise_t[:, sl],
                                 start=False, stop=True)
                eh = envgs[s // 3]
                el = slice((s % 3) * N, (s % 3 + 1) * N)
                nc.vector.tensor_tensor(sig[:, sl], ps[:], eh[:, el], OP.mult)
                nc.vector.tensor_reduce(mxcols[:, s:s + 1], sig[:, sl],
                                        axis=mybir.AxisListType.X, op=OP.max,
                                        apply_absolute_value=True)

            # ---- per-batch max: fold -> transpose -> j-fold -> bcast ----
            mx = smpool.tile([128, 1], F16, tag="mx")
            nc.vector.tensor_reduce(mx[:], mxcols[:], axis=mybir.AxisListType.X,
                                    op=OP.max)
            mxT = pbpool.tile([1, 128], F16, tag="ps2", name="mxT")
            nc.tensor.transpose(mxT[:], mx[:], identp[:])
            row2 = smpool.tile([1, 2], F32, tag="row2")
            nc.vector.tensor_reduce(row2[:],
                                    mxT[:].rearrange("o (b j) -> o b j", j=J),
                                    axis=mybir.AxisListType.X, op=OP.max)
            nc.vector.tensor_scalar(row2[:], row2[:], 1e-5, None, OP.add)
            inv2 = smpool.tile([1, 2], F32, tag="inv2")
            nc.vector.reciprocal(inv2[:], row2[:])
            invrow = smpool.tile([1, 128], F32, tag="invrow")
            nc.vector.tensor_copy(
                invrow[:].rearrange("o (b j) -> o b j", j=J),
                inv2[:].rearrange("o (b u) -> o b u", u=1).broadcast_to(
                    [1, BL, J]))
            invp = pbpool.tile([128, 1], F32, tag="ps2", name="invp")
            nc.tensor.matmul(invp[:], invrow[:], consts[0:1, 13:14],
                             start=True, stop=True)
            inv = smpool.tile([128, 1], F32, tag="inv")
            nc.vector.tensor_copy(inv[:], invp[:])

            # ---- normalize + store (DVE half || ACT half, 2 DMAs) ----
            nc.vector.tensor_scalar(outn[:, 0:H2], sig[:, 0:H2], inv[:],
                                    None, OP.mult)
            nc.sync.dma_start(out_d[:, 0:H2], outn[:, 0:H2])
            nc.vector.tensor_scalar(outn[:, H2:L], sig[:, H2:L], inv[:],
                                    None, OP.mult)
            nc.sync.dma_start(out_d[:, H2:L], outn[:, H2:L])

    nc.finalize()
    return nc


def _host_prep(harmonic_dist, noise_bands, adsr, gain, noise):
    """Weights/consts in f64 (cast f32/f16 at the end); the angle split is
    sin(theta_jk + phi_kn) with both angles exact in f64."""
    step64 = np.float64(f32(np.float64(T / SR) / (T - 1)))
    k = np.arange(1, NH + 1, dtype=f32)
    ck64 = (f32(2.0 * np.pi * 440.0) * k).astype(np.float64)
    n = np.arange(L, dtype=np.float64)
    jj = np.arange(J, dtype=np.float64)
    phi = ck64[:, None] * (step64 * n[None, :])          # (64, L)
    theta = ck64[:, None] * (step64 * (jj[None, :] * L))  # (64, J)
    tab = np.concatenate([np.cos(phi), np.sin(phi)], axis=0).astype(f16)
    sinth, costh = np.sin(theta), np.cos(theta)          # (64, J)

    A = np.ascontiguousarray(harmonic_dist, dtype=f32).astype(np.float64)
    identp = np.eye(128, dtype=f16)

    # noise shipped as (noise - 0.5) fp16; the 2*lev scale rides in the
    # per-batch scaled identity so no on-device nsig pass is needed
    npad = np.zeros((B, TPAD), f16)
    npad[:, :T] = (noise.astype(f32) - f32(0.5)).astype(f16)

    # ADSR int constants, replicating reference rounding exactly
    att_in, dec_in, sus, rel_in = (adsr[:, 0].astype(f32), adsr[:, 1].astype(f32),
                                   adsr[:, 2].astype(f32), adsr[:, 3].astype(f32))
    a = np.floor((att_in * f32(0.5)) * f32(SR)).astype(np.int64) + 1
    d = np.floor((dec_in * f32(0.5)) * f32(SR)).astype(np.int64) + 1
    r = np.floor((rel_in * f32(0.5)) * f32(SR)).astype(np.int64) + 1
    total = a + d + r
    scale = (f32(T) / total.astype(f32)).astype(f32)
    resc = total > T
    a = np.where(resc, np.floor(a.astype(f32) * scale).astype(np.int64), a)
    d = np.where(resc, np.floor(d.astype(f32) * scale).astype(np.int64), d)
    r = np.where(resc, np.floor(r.astype(f32) * scale).astype(np.int64), r)
    s = np.maximum(T - (a + d + r), 0)

    g64 = gain.astype(np.float64)[:, 0]
    sus64 = sus.astype(np.float64)
    m_a = np.maximum(a - 1, 1).astype(np.float64)
    m_d = np.maximum(d - 1, 1).astype(np.float64)
    m_r = np.maximum(r - 1, 1).astype(np.float64)
    A2 = (a + d + s).astype(np.float64)
    lev64 = (np.mean(noise_bands.astype(f32), axis=1, dtype=f32)
             * f32(0.1)).astype(np.float64)

    in_maps = []
    for c in range(NCORES):
        noise_c = np.ascontiguousarray(
            npad[2 * c:2 * c + 2].reshape(128, L))

        sident = np.zeros((128, 128), np.float64)
        for p in range(128):
            sident[p, p] = 2.0 * lev64[2 * c + p // J]

        wmat = np.zeros((128, 128), np.float64)
        for bl in range(BL):
            b = 2 * c + bl
            # W[k, p] = A[b,k]*sin(theta[k,j]); W[64+k, p] = A[b,k]*cos(..)
            wmat[:NH, bl * J:(bl + 1) * J] = A[b][:, None] * sinth
            wmat[NH:, bl * J:(bl + 1) * J] = A[b][:, None] * costh

        consts = np.zeros((128, 16), np.float64)
        consts[:, 13] = 1.0
        for bl in range(BL):
            b = 2 * c + bl
            for j in range(J):
                p = bl * J + j
                base = j * L
                sc_att = g64[b] / m_a[b]
                consts[p, 0] = sc_att
                consts[p, 1] = f32(sc_att) * np.float64(base)
                consts[p, 2] = f32(sc_att) * np.float64(base + H2)
                sc_z = -1.0 / m_d[b]
                consts[p, 3] = sc_z
                consts[p, 4] = 1.0 - (base - a[b]) / m_d[b]
                consts[p, 5] = 1.0 - (base + H2 - a[b]) / m_d[b]
                consts[p, 6] = (1.0 - sus64[b]) * g64[b]
                consts[p, 7] = sus64[b] * g64[b]
                sc_w = sus64[b] * g64[b] / m_r[b]
                consts[p, 8] = sc_w
                consts[p, 9] = -f32(sc_w) * (A2[b] - np.float64(base))
                consts[p, 10] = -f32(sc_w) * (A2[b] - np.float64(base + H2))
                consts[p, 11] = 2.0 * lev64[b]
                consts[p, 12] = -lev64[b]
        in_maps.append({
            "tab": tab,
            "wmat": wmat.astype(f16),
            "ident": sident.astype(f16),
            "identp": identp,
            "noise_p": noise_c,
            "consts": consts.astype(f32),
        })
    return in_maps


LAST_RESULTS = None


def kernel(base_audio, harmonic_dist, noise_bands, adsr, gain, noise):
    global LAST_RESULTS
    if "nc" not in _cache:
        _cache["nc"] = _build_nc()
    nc = _cache["nc"]

    in_maps = _host_prep(
        np.asarray(harmonic_dist), np.asarray(noise_bands),
        np.asarray(adsr), np.asarray(gain), np.asarray(noise))

    trace = bool(os.environ.get("KERNEL_TRACE"))
    res = run_bass_kernel_spmd(nc, in_maps, list(range(NCORES)), trace=trace)
    LAST_RESULTS = res

    out = np.empty((B, TPAD), f32)
    for c in range(NCORES):
        out[2 * c:2 * c + 2] = (res.results[c]["out_sig"]
                                .astype(f32).reshape(BL, TPAD))
    return np.ascontiguousarray(out[:, :T])
